# revision 18
# baseline (speedup 1.0000x reference)
"""Trainium2 Bass kernel for a chain of 20 radial flows on [8388608, 4] fp32.

Strategy: pure data parallel over 8 NeuronCores. Each core gets B/8 samples,
host-transposed to [4, S] so components sit on separate SBUF partitions
(partition 32*c + b holds component c of sample-block b).

Per flow k (sequential chain):
  d   = v + delta_{k-1}            (delta folded as per-partition ACT bias)
  sq  = d*d                        ACT Square (bf16 out)
  r2  = sum_c sq[c]                PE matmul with 0/1 bf16 stationary,
                                   output *packed* onto all 128 partitions
  r   = sqrt(r2)                   ACT
  ws  = r/beta_k + alpha_k/beta_k  ACT Identity (AP scale+bias)
  m   = 1 + 1/ws                   one custom DVE op (RADIAL_M): quadratic
                                   Chebyshev seed on x*bitcast(~x), +1 via
                                   a per-partition ones column (~5e-5 rel)
  m_b = broadcast(m)               PE matmul with 0/1 stationary -> PSUM
  v  <- (v + delta_{k-1}) * m_b    DVE scalar_tensor_tensor (fused)
Final: out = v + x0[19].
"""

import sys

if "/opt/trn_rl_repo" not in sys.path:
    sys.path.insert(0, "/opt/trn_rl_repo")

from contextlib import ExitStack

import numpy as np

import concourse.bass as bass
import concourse.tile as tile
from concourse import bacc, mybir

F32 = mybir.dt.float32
F32R = mybir.dt.float32r
BF16 = mybir.dt.bfloat16

BATCH = 8388608
DIM = 4
N_FLOWS = 20
N_CORES = 8
S = BATCH // N_CORES          # samples per core
NB = 32                       # sample-blocks (per quadrant of partitions)
FC = 2048                     # chunk free-dim (columns) per tile
NT = S // NB // FC            # chunks per core
PK = FC // 4                  # packed (per-sample) free-dim per chunk

_CACHE = {}

# Chebyshev-minimax quadratic seed for 1/x on the u = x*bitcast(~x) interval
# [-4.5, -4]: 1/x ~= bitcast(~x) * (c2*u^2 + c1*u + c0), max rel err 5.2e-5.
_RM_C = (-0.706758876, -0.166355887, -0.013040724)


def _radial_m_op():
    """out = bitcast(~x)*((C2*u + C1)*u + C0) + src1, u = x*bitcast(~x).

    With src1 = ones column: out = 1 + 1/Src0 to ~5e-5 — fuses the
    reciprocal and the +1 of m = 1 + beta/(alpha+r) into one 8-stage op."""
    from concourse import dve_ops
    from concourse.dve_spec import AluOp, Bin, C0, C1, C2, Spec, Src0, Src1, lower
    from concourse.dve_uop import DveOpSpec

    for op in dve_ops.OPS:
        if op.name == "RADIAL_M":
            return op
    _not = Bin(AluOp.BITWISE_NOT, Src0, Src0)
    _u = Src0 * _not
    body = _not * ((_u * C2 + C1) * _u + C0) + Src1

    def ref(in0, in1, c0, c1, c2):
        not_x = (~in0.view(np.int32)).view(np.float32)
        u = (in0 * not_x).astype(np.float32)
        return (not_x * ((u * c2 + c1) * u + c0) + in1).astype(np.float32)

    spec = Spec(body=body, reference=ref)
    row = max(dve_ops._SUB_OPCODE_FOR_NAME.values()) + 1
    assert row < 0x20
    dve_ops._SUB_OPCODE_FOR_NAME["RADIAL_M"] = row
    shas = {}
    for ver in ("v3", "v4"):
        uops = lower(spec, ver=ver)
        shas[ver] = DveOpSpec(
            name="RADIAL_M", opcode=row, uops=uops, rd1_en=True).sha(ver)
    op = dve_ops.DveOp("RADIAL_M", spec, subdim=False, uops_sha=shas)
    dve_ops.OPS.append(op)
    dve_ops.CUSTOM_DVE_SPECS["RADIAL_M"] = spec
    return op


def _build_program(n_flows=N_FLOWS, nt=NT):
    nc = bacc.Bacc("TRN2", target_bir_lowering=False, debug=False,
                   num_devices=N_CORES)
    s = nt * NB * FC
    xt = nc.dram_tensor("xt", [DIM, s], F32, kind="ExternalInput").ap()
    wr = nc.dram_tensor("wr", [4, 128, 128], BF16, kind="ExternalInput").ap()
    wb = nc.dram_tensor("wb", [4, 128, 128], F32R, kind="ExternalInput").ap()
    pr = nc.dram_tensor("pr", [128, 3 * N_FLOWS + 2], F32,
                        kind="ExternalInput").ap()
    ot = nc.dram_tensor("ot", [DIM, s], F32, kind="ExternalOutput").ap()

    # DRAM view: partition (c*32+b) <- comp c, block b; free (t, f)
    xt_r = xt.rearrange("c (b t f) -> (c b) t f", b=NB, f=FC)
    ot_r = ot.rearrange("c (b t f) -> (c b) t f", b=NB, f=FC)

    AL = mybir.AluOpType
    ACT = mybir.ActivationFunctionType

    def col(i):  # per-partition scalar AP from the params tile
        return pr_t[:, i:i + 1]

    with tile.TileContext(nc) as tc, ExitStack() as ctx:
        singles = ctx.enter_context(tc.tile_pool(name="singles", bufs=1))
        state = ctx.enter_context(tc.tile_pool(name="state", bufs=1))
        sq_pool = ctx.enter_context(tc.tile_pool(name="sq", bufs=4))
        pk_pool = ctx.enter_context(tc.tile_pool(name="pk", bufs=6))
        r2_pool = ctx.enter_context(
            tc.tile_pool(name="r2", bufs=2, space="PSUM"))
        mb_pool = ctx.enter_context(
            tc.tile_pool(name="mb", bufs=3, space="PSUM"))

        pr_t = singles.tile([128, 3 * N_FLOWS + 2], F32)
        nc.sync.dma_start(pr_t[:], pr[:])
        wr_t = singles.tile([128, 4, 128], BF16)
        nc.sync.dma_start(wr_t[:], wr.rearrange("j k m -> k j m"))
        wb_t = singles.tile([128, 4, 128], F32R)
        nc.sync.dma_start(wb_t[:], wb.rearrange("j k m -> k j m"))
        ones_t = singles.tile([128, PK], F32)
        nc.vector.memset(ones_t[:], 1.0)

        v = state.tile([128, nt * FC], F32)
        for t in range(nt):
            nc.sync.dma_start(v[:, bass.ts(t, FC)], xt_r[:, t, :])

        for k in range(n_flows):
            c_dprev, c_invb, c_aob = k, N_FLOWS + 1 + k, 2 * N_FLOWS + 1 + k
            for t in range(nt):
                vchunk = v[:, bass.ts(t, FC)]
                sq = sq_pool.tile([128, FC], BF16)
                nc.scalar.activation(sq[:], vchunk, ACT.Square,
                                     bias=col(c_dprev), scale=1.0)
                r2 = r2_pool.tile([128, PK], F32)
                for j in range(4):
                    nc.tensor.matmul(
                        out=r2[:],
                        lhsT=wr_t[:, j, :],
                        rhs=sq[:, bass.ts(j, PK)],
                        start=(j == 0), stop=(j == 3))
                r = pk_pool.tile([128, PK], F32, tag="r")
                nc.scalar.activation(r[:], r2[:], ACT.Sqrt)
                w = pk_pool.tile([128, PK], F32, tag="w")
                if k % 2 == 0:
                    nc.scalar.activation(w[:], r[:], ACT.Identity,
                                         bias=col(c_aob), scale=col(c_invb))
                else:
                    nc.vector.tensor_scalar(w[:], r[:], col(c_invb),
                                            col(c_aob), AL.mult, AL.add)
                m = pk_pool.tile([128, PK], F32R, tag="m")
                nc.vector._custom_dve(
                    _radial_m_op(), out=m[:], in0=w[:],
                    in1=ones_t[:], s0=_RM_C[0], s1=_RM_C[1],
                    imm2=_RM_C[2])
                for h in range(2):  # two PSUM half-tiles so bcast/stt overlap
                    mb = mb_pool.tile([128, FC // 2], F32)
                    for jj in range(2):
                        j = 2 * h + jj
                        nc.tensor.matmul(
                            out=mb[:, bass.ts(jj, PK)],
                            lhsT=wb_t[:, j, :],
                            rhs=m[:], start=True, stop=True)
                    vh = v[:, t * FC + h * (FC // 2):
                           t * FC + (h + 1) * (FC // 2)]
                    nc.vector.scalar_tensor_tensor(
                        out=vh, in0=vh, scalar=col(c_dprev), in1=mb[:],
                        op0=AL.add, op1=AL.mult)

        for t in range(nt):
            vchunk = v[:, bass.ts(t, FC)]
            nc.vector.tensor_scalar(vchunk, vchunk, col(N_FLOWS), None, AL.add)
            nc.sync.dma_start(ot_r[:, t, :], vchunk)

    nc.compile()
    return nc


def _host_params(x0s, alpha_primes, beta_primes, n_flows=N_FLOWS):
    x0s = np.asarray(x0s, np.float32)
    sp_a = np.logaddexp(np.float32(0.0), np.asarray(alpha_primes, np.float32))
    sp_b = np.logaddexp(np.float32(0.0), np.asarray(beta_primes, np.float32))
    alpha = sp_a.astype(np.float32)
    beta = (-alpha + sp_b).astype(np.float32)

    # params: dprev[k], final, invbeta[k], alpha/beta[k], ones
    pr = np.zeros((128, 3 * N_FLOWS + 2), np.float32)
    pr[:, 3 * N_FLOWS + 1] = 1.0
    comp = np.arange(128) // 32  # component index per partition
    for k in range(n_flows):
        dprev = -x0s[0] if k == 0 else x0s[k - 1] - x0s[k]
        bk = beta[k] if beta[k] != 0.0 else np.float32(1e-30)
        pr[:, k] = dprev[comp]
        pr[:, N_FLOWS + 1 + k] = 1.0 / bk
        pr[:, 2 * N_FLOWS + 1 + k] = alpha[k] / bk
    pr[:, N_FLOWS] = x0s[n_flows - 1][comp]

    # stationaries: wr reduce (comp partitions -> packed), wb broadcast (K=32)
    import ml_dtypes
    wr = np.zeros((4, 128, 128), np.float32)
    wb = np.zeros((4, 128, 128), np.float32)
    b = np.arange(NB)
    for j in range(4):
        for c in range(4):
            wr[j, 32 * c + b, 32 * j + b] = 1.0
            wb[j, 32 * j + b, 32 * c + b] = 1.0
    return pr, wr.astype(ml_dtypes.bfloat16), wb


def kernel(X, x0s, alpha_primes, beta_primes):
    from concourse.bass_utils import run_bass_kernel_spmd

    X = np.asarray(X, np.float32)
    pr, wr, wb = _host_params(x0s, alpha_primes, beta_primes)

    if "nc" not in _CACHE:
        _CACHE["nc"] = _build_program()
    nc = _CACHE["nc"]

    in_maps = []
    for c in range(N_CORES):
        shard = X[c * S:(c + 1) * S]
        in_maps.append({
            "xt": np.ascontiguousarray(shard.T),
            "wr": wr,
            "wb": wb,
            "pr": pr,
        })
    res = run_bass_kernel_spmd(nc, in_maps, list(range(N_CORES)))
    out = np.empty((BATCH, DIM), np.float32)
    for c in range(N_CORES):
        out[c * S:(c + 1) * S] = res.results[c]["ot"].T
    return out


# revision 20
# speedup vs baseline: 1.0234x; 1.0234x over previous
"""Trainium2 Bass kernel for a chain of 20 radial flows on [8388608, 4] fp32.

Strategy: pure data parallel over 8 NeuronCores. Each core gets B/8 samples,
host-transposed to [4, S] so components sit on separate SBUF partitions
(partition 32*c + b holds component c of sample-block b).

Per flow k (sequential chain):
  d   = v + delta_{k-1}            (delta folded as per-partition ACT bias)
  sq  = d*d                        ACT Square (bf16 out)
  r2  = sum_c sq[c]                PE matmul with 0/1 bf16 stationary,
                                   output *packed* onto all 128 partitions
  r   = sqrt(r2)                   ACT
  ws  = r/beta_k + alpha_k/beta_k  ACT Identity (AP scale+bias)
  m   = 1 + 1/ws                   one custom DVE op (RADIAL_M): quadratic
                                   Chebyshev seed on x*bitcast(~x), +1 via
                                   a per-partition ones column (~5e-5 rel)
  m_b = broadcast(m)               PE matmul with 0/1 stationary -> PSUM
  v  <- (v + delta_{k-1}) * m_b    DVE scalar_tensor_tensor (fused)
Final: out = v + x0[19].
"""

import sys

if "/opt/trn_rl_repo" not in sys.path:
    sys.path.insert(0, "/opt/trn_rl_repo")

from contextlib import ExitStack

import numpy as np

import concourse.bass as bass
import concourse.tile as tile
from concourse import bacc, mybir

F32 = mybir.dt.float32
F32R = mybir.dt.float32r
BF16 = mybir.dt.bfloat16

BATCH = 8388608
DIM = 4
N_FLOWS = 20
N_CORES = 8
S = BATCH // N_CORES          # samples per core
NB = 32                       # sample-blocks (per quadrant of partitions)
FC = 2048                     # chunk free-dim (columns) per tile
NT = S // NB // FC            # chunks per core
PK = FC // 4                  # packed (per-sample) free-dim per chunk

_CACHE = {}

# Chebyshev-minimax quadratic seed for 1/x on the u = x*bitcast(~x) interval
# [-4.5, -4]: 1/x ~= bitcast(~x) * (c2*u^2 + c1*u + c0), max rel err 5.2e-5.
_RM_C = (-0.706758876, -0.166355887, -0.013040724)


def _radial_m_op():
    """out = bitcast(~x)*((C2*u + C1)*u + C0) + src1, u = x*bitcast(~x).

    With src1 = ones column: out = 1 + 1/Src0 to ~5e-5 — fuses the
    reciprocal and the +1 of m = 1 + beta/(alpha+r) into one 8-stage op."""
    from concourse import dve_ops
    from concourse.dve_spec import AluOp, Bin, C0, C1, C2, Spec, Src0, Src1, lower
    from concourse.dve_uop import DveOpSpec

    for op in dve_ops.OPS:
        if op.name == "RADIAL_M":
            return op
    _not = Bin(AluOp.BITWISE_NOT, Src0, Src0)
    _u = Src0 * _not
    body = _not * ((_u * C2 + C1) * _u + C0) + Src1

    def ref(in0, in1, c0, c1, c2):
        not_x = (~in0.view(np.int32)).view(np.float32)
        u = (in0 * not_x).astype(np.float32)
        return (not_x * ((u * c2 + c1) * u + c0) + in1).astype(np.float32)

    spec = Spec(body=body, reference=ref)
    row = max(dve_ops._SUB_OPCODE_FOR_NAME.values()) + 1
    assert row < 0x20
    dve_ops._SUB_OPCODE_FOR_NAME["RADIAL_M"] = row
    shas = {}
    for ver in ("v3", "v4"):
        uops = lower(spec, ver=ver)
        shas[ver] = DveOpSpec(
            name="RADIAL_M", opcode=row, uops=uops, rd1_en=True).sha(ver)
    op = dve_ops.DveOp("RADIAL_M", spec, subdim=False, uops_sha=shas)
    dve_ops.OPS.append(op)
    dve_ops.CUSTOM_DVE_SPECS["RADIAL_M"] = spec
    return op


def _build_program(n_flows=N_FLOWS, nt=NT):
    nc = bacc.Bacc("TRN2", target_bir_lowering=False, debug=False,
                   num_devices=N_CORES)
    s = nt * NB * FC
    xt = nc.dram_tensor("xt", [DIM, s], F32, kind="ExternalInput").ap()
    wr = nc.dram_tensor("wr", [4, 128, 128], BF16, kind="ExternalInput").ap()
    wb = nc.dram_tensor("wb", [4, 128, 128], F32R, kind="ExternalInput").ap()
    pr = nc.dram_tensor("pr", [128, 3 * N_FLOWS + 2], F32,
                        kind="ExternalInput").ap()
    ot = nc.dram_tensor("ot", [DIM, s], F32, kind="ExternalOutput").ap()

    # DRAM view: partition (c*32+b) <- comp c, block b; free (t, f)
    xt_r = xt.rearrange("c (b t f) -> (c b) t f", b=NB, f=FC)
    ot_r = ot.rearrange("c (b t f) -> (c b) t f", b=NB, f=FC)

    AL = mybir.AluOpType
    ACT = mybir.ActivationFunctionType

    def col(i):  # per-partition scalar AP from the params tile
        return pr_t[:, i:i + 1]

    with tile.TileContext(nc) as tc, ExitStack() as ctx:
        singles = ctx.enter_context(tc.tile_pool(name="singles", bufs=1))
        state = ctx.enter_context(tc.tile_pool(name="state", bufs=1))
        sq_pool = ctx.enter_context(tc.tile_pool(name="sq", bufs=4))
        pk_pool = ctx.enter_context(tc.tile_pool(name="pk", bufs=6))
        r2_pool = ctx.enter_context(
            tc.tile_pool(name="r2", bufs=2, space="PSUM"))
        mb_pool = ctx.enter_context(
            tc.tile_pool(name="mb", bufs=3, space="PSUM"))

        pr_t = singles.tile([128, 3 * N_FLOWS + 2], F32)
        nc.sync.dma_start(pr_t[:], pr[:])
        wr_t = singles.tile([128, 4, 128], BF16)
        nc.sync.dma_start(wr_t[:], wr.rearrange("j k m -> k j m"))
        wb_t = singles.tile([128, 4, 128], F32R)
        nc.sync.dma_start(wb_t[:], wb.rearrange("j k m -> k j m"))
        ones_t = singles.tile([128, PK], F32)
        nc.vector.memset(ones_t[:], 1.0)

        v = state.tile([128, nt * FC], F32)
        for t in range(nt):
            nc.sync.dma_start(v[:, bass.ts(t, FC)], xt_r[:, t, :])

        for k in range(n_flows):
            c_dprev, c_invb, c_aob = k, N_FLOWS + 1 + k, 2 * N_FLOWS + 1 + k
            for t in range(nt):
                vchunk = v[:, bass.ts(t, FC)]
                sq = sq_pool.tile([128, FC], BF16)
                nc.scalar.activation(sq[:], vchunk, ACT.Square,
                                     bias=col(c_dprev), scale=1.0)
                r2 = r2_pool.tile([128, PK], F32)
                for j in range(4):
                    nc.tensor.matmul(
                        out=r2[:],
                        lhsT=wr_t[:, j, :],
                        rhs=sq[:, bass.ts(j, PK)],
                        start=(j == 0), stop=(j == 3))
                r = pk_pool.tile([128, PK], F32, tag="r")
                nc.scalar.activation(r[:], r2[:], ACT.Sqrt)
                w = pk_pool.tile([128, PK], F32, tag="w")
                nc.scalar.activation(w[:], r[:], ACT.Identity,
                                     bias=col(c_aob), scale=col(c_invb))
                m = pk_pool.tile([128, PK], F32R, tag="m")
                nc.vector._custom_dve(
                    _radial_m_op(), out=m[:], in0=w[:],
                    in1=ones_t[:], s0=_RM_C[0], s1=_RM_C[1],
                    imm2=_RM_C[2])
                for h in range(2):  # two PSUM half-tiles so bcast/stt overlap
                    mb = mb_pool.tile([128, FC // 2], F32)
                    for jj in range(2):
                        j = 2 * h + jj
                        nc.tensor.matmul(
                            out=mb[:, bass.ts(jj, PK)],
                            lhsT=wb_t[:, j, :],
                            rhs=m[:], start=True, stop=True)
                    vh = v[:, t * FC + h * (FC // 2):
                           t * FC + (h + 1) * (FC // 2)]
                    nc.vector.scalar_tensor_tensor(
                        out=vh, in0=vh, scalar=col(c_dprev), in1=mb[:],
                        op0=AL.add, op1=AL.mult)
                if k == n_flows - 1:
                    nc.vector.tensor_scalar(vchunk, vchunk, col(N_FLOWS),
                                            None, AL.add)
                    nc.sync.dma_start(ot_r[:, t, :], vchunk)

    nc.compile()
    return nc


def _host_params(x0s, alpha_primes, beta_primes, n_flows=N_FLOWS):
    x0s = np.asarray(x0s, np.float32)
    sp_a = np.logaddexp(np.float32(0.0), np.asarray(alpha_primes, np.float32))
    sp_b = np.logaddexp(np.float32(0.0), np.asarray(beta_primes, np.float32))
    alpha = sp_a.astype(np.float32)
    beta = (-alpha + sp_b).astype(np.float32)

    # params: dprev[k], final, invbeta[k], alpha/beta[k], ones
    pr = np.zeros((128, 3 * N_FLOWS + 2), np.float32)
    pr[:, 3 * N_FLOWS + 1] = 1.0
    comp = np.arange(128) // 32  # component index per partition
    for k in range(n_flows):
        dprev = -x0s[0] if k == 0 else x0s[k - 1] - x0s[k]
        bk = beta[k] if beta[k] != 0.0 else np.float32(1e-30)
        pr[:, k] = dprev[comp]
        pr[:, N_FLOWS + 1 + k] = 1.0 / bk
        pr[:, 2 * N_FLOWS + 1 + k] = alpha[k] / bk
    pr[:, N_FLOWS] = x0s[n_flows - 1][comp]

    # stationaries: wr reduce (comp partitions -> packed), wb broadcast (K=32)
    import ml_dtypes
    wr = np.zeros((4, 128, 128), np.float32)
    wb = np.zeros((4, 128, 128), np.float32)
    b = np.arange(NB)
    for j in range(4):
        for c in range(4):
            wr[j, 32 * c + b, 32 * j + b] = 1.0
            wb[j, 32 * j + b, 32 * c + b] = 1.0
    return pr, wr.astype(ml_dtypes.bfloat16), wb


def kernel(X, x0s, alpha_primes, beta_primes):
    from concourse.bass_utils import run_bass_kernel_spmd

    X = np.asarray(X, np.float32)
    pr, wr, wb = _host_params(x0s, alpha_primes, beta_primes)

    if "nc" not in _CACHE:
        _CACHE["nc"] = _build_program()
    nc = _CACHE["nc"]

    in_maps = []
    for c in range(N_CORES):
        shard = X[c * S:(c + 1) * S]
        in_maps.append({
            "xt": np.ascontiguousarray(shard.T),
            "wr": wr,
            "wb": wb,
            "pr": pr,
        })
    res = run_bass_kernel_spmd(nc, in_maps, list(range(N_CORES)))
    out = np.empty((BATCH, DIM), np.float32)
    for c in range(N_CORES):
        out[c * S:(c + 1) * S] = res.results[c]["ot"].T
    return out


# revision 22
# speedup vs baseline: 1.1102x; 1.0848x over previous
"""Trainium2 Bass kernel for a chain of 20 radial flows on [8388608, 4] fp32.

Strategy: pure data parallel over 8 NeuronCores. Each core gets B/8 samples,
host-transposed to [4, S] so components sit on separate SBUF partitions
(partition 32*c + b holds component c of sample-block b).

Per flow k (sequential chain):
  d   = v + delta_{k-1}            (delta folded as per-partition ACT bias)
  sq  = d*d                        ACT Square (bf16 out)
  r2  = sum_c sq[c]                PE matmul with 0/1 bf16 stationary,
                                   output *packed* onto all 128 partitions
  r   = sqrt(r2)                   ACT
  ws  = r/beta_k + alpha_k/beta_k  ACT Identity (AP scale+bias)
  m   = 1 + 1/ws                   one custom DVE op (RADIAL_M): quadratic
                                   Chebyshev seed on x*bitcast(~x), +1 via
                                   a per-partition ones column (~5e-5 rel)
  m_b = broadcast(m)               PE matmul with 0/1 stationary -> PSUM
  v  <- (v + delta_{k-1}) * m_b    DVE scalar_tensor_tensor (fused)
Final: out = v + x0[19].
"""

import sys

if "/opt/trn_rl_repo" not in sys.path:
    sys.path.insert(0, "/opt/trn_rl_repo")

from contextlib import ExitStack

import numpy as np

import concourse.bass as bass
import concourse.tile as tile
from concourse import bacc, mybir

F32 = mybir.dt.float32
F32R = mybir.dt.float32r
BF16 = mybir.dt.bfloat16

BATCH = 8388608
DIM = 4
N_FLOWS = 20
N_CORES = 8
S = BATCH // N_CORES          # samples per core
NB = 32                       # sample-blocks (per quadrant of partitions)
FC = 2048                     # chunk free-dim (columns) per tile
NT = S // NB // FC            # chunks per core
PK = FC // 4                  # packed (per-sample) free-dim per chunk

_CACHE = {}

# Chebyshev-minimax quadratic seed for 1/x on the u = x*bitcast(~x) interval
# [-4.5, -4]: 1/x ~= bitcast(~x) * (c2*u^2 + c1*u + c0), max rel err 5.2e-5.
_RM_C = (-0.706758876, -0.166355887, -0.013040724)


def _radial_m_op():
    """out = bitcast(~x)*((C2*u + C1)*u + C0) + src1, u = x*bitcast(~x).

    With src1 = ones column: out = 1 + 1/Src0 to ~5e-5 — fuses the
    reciprocal and the +1 of m = 1 + beta/(alpha+r) into one 8-stage op."""
    from concourse import dve_ops
    from concourse.dve_spec import AluOp, Bin, C0, C1, C2, Spec, Src0, Src1, lower
    from concourse.dve_uop import DveOpSpec

    for op in dve_ops.OPS:
        if op.name == "RADIAL_M":
            return op
    _not = Bin(AluOp.BITWISE_NOT, Src0, Src0)
    _u = Src0 * _not
    body = _not * ((_u * C2 + C1) * _u + C0) + Src1

    def ref(in0, in1, c0, c1, c2):
        not_x = (~in0.view(np.int32)).view(np.float32)
        u = (in0 * not_x).astype(np.float32)
        return (not_x * ((u * c2 + c1) * u + c0) + in1).astype(np.float32)

    spec = Spec(body=body, reference=ref)
    row = max(dve_ops._SUB_OPCODE_FOR_NAME.values()) + 1
    assert row < 0x20
    dve_ops._SUB_OPCODE_FOR_NAME["RADIAL_M"] = row
    shas = {}
    for ver in ("v3", "v4"):
        uops = lower(spec, ver=ver)
        shas[ver] = DveOpSpec(
            name="RADIAL_M", opcode=row, uops=uops, rd1_en=True).sha(ver)
    op = dve_ops.DveOp("RADIAL_M", spec, subdim=False, uops_sha=shas)
    dve_ops.OPS.append(op)
    dve_ops.CUSTOM_DVE_SPECS["RADIAL_M"] = spec
    return op


def _build_program(n_flows=N_FLOWS, nt=NT):
    nc = bacc.Bacc("TRN2", target_bir_lowering=False, debug=False,
                   num_devices=N_CORES)
    s = nt * NB * FC
    xt = nc.dram_tensor("xt", [DIM, s], F32, kind="ExternalInput").ap()
    wr = nc.dram_tensor("wr", [4, 128, 128], BF16, kind="ExternalInput").ap()
    wb = nc.dram_tensor("wb", [4, 128, 128], F32R, kind="ExternalInput").ap()
    pr = nc.dram_tensor("pr", [128, 3 * N_FLOWS + 2], F32,
                        kind="ExternalInput").ap()
    ot = nc.dram_tensor("ot", [DIM, s], F32, kind="ExternalOutput").ap()

    # DRAM view: partition (c*32+b) <- comp c, block b; free (t, f)
    xt_r = xt.rearrange("c (b t f) -> (c b) t f", b=NB, f=FC)
    ot_r = ot.rearrange("c (b t f) -> (c b) t f", b=NB, f=FC)

    AL = mybir.AluOpType
    ACT = mybir.ActivationFunctionType

    def col(i):  # per-partition scalar AP from the params tile
        return pr_t[:, i:i + 1]

    with tile.TileContext(nc) as tc, ExitStack() as ctx:
        singles = ctx.enter_context(tc.tile_pool(name="singles", bufs=1))
        state = ctx.enter_context(tc.tile_pool(name="state", bufs=1))
        sq_pool = ctx.enter_context(tc.tile_pool(name="sq", bufs=4))
        pk_pool = ctx.enter_context(tc.tile_pool(name="pk", bufs=4))
        r2_pool = ctx.enter_context(
            tc.tile_pool(name="r2", bufs=2, space="PSUM"))
        mb_pool = ctx.enter_context(
            tc.tile_pool(name="mb", bufs=2, space="PSUM"))

        pr_t = singles.tile([128, 3 * N_FLOWS + 2], F32)
        nc.sync.dma_start(pr_t[:], pr[:])
        wr_t = singles.tile([128, 4, 128], BF16)
        nc.sync.dma_start(wr_t[:], wr.rearrange("j k m -> k j m"))
        wb_t = singles.tile([128, 4, 128], F32R)
        nc.sync.dma_start(wb_t[:], wb.rearrange("j k m -> k j m"))
        ones_t = singles.tile([128, 2 * PK], F32)
        nc.vector.memset(ones_t[:], 1.0)

        v = state.tile([128, nt * FC], F32)
        for t in range(nt):
            nc.sync.dma_start(v[:, bass.ts(t, FC)], xt_r[:, t, :])

        for k in range(n_flows):
            c_dprev, c_invb, c_aob = k, N_FLOWS + 1 + k, 2 * N_FLOWS + 1 + k
            for tp in range(nt // 2):
                # chunk pair shares one packed s-chain (amortized op inits)
                r2 = r2_pool.tile([128, 2 * PK], F32)
                for u in range(2):
                    t = 2 * tp + u
                    vchunk = v[:, bass.ts(t, FC)]
                    sq = sq_pool.tile([128, FC], BF16)
                    nc.scalar.activation(sq[:], vchunk, ACT.Square,
                                         bias=col(c_dprev), scale=1.0)
                    for j in range(4):
                        nc.tensor.matmul(
                            out=r2[:, u * PK:(u + 1) * PK],
                            lhsT=wr_t[:, j, :],
                            rhs=sq[:, bass.ts(j, PK)],
                            start=(j == 0), stop=(j == 3))
                r = pk_pool.tile([128, 2 * PK], F32, tag="r")
                nc.scalar.activation(r[:], r2[:], ACT.Sqrt)
                w = pk_pool.tile([128, 2 * PK], F32, tag="w")
                nc.scalar.activation(w[:], r[:], ACT.Identity,
                                     bias=col(c_aob), scale=col(c_invb))
                m = pk_pool.tile([128, 2 * PK], F32R, tag="m")
                nc.vector._custom_dve(
                    _radial_m_op(), out=m[:], in0=w[:],
                    in1=ones_t[:], s0=_RM_C[0], s1=_RM_C[1],
                    imm2=_RM_C[2])
                for u in range(2):
                    t = 2 * tp + u
                    vchunk = v[:, bass.ts(t, FC)]
                    for h in range(2):
                        mb = mb_pool.tile([128, FC // 2], F32)
                        for jj in range(2):
                            j = 2 * h + jj
                            nc.tensor.matmul(
                                out=mb[:, bass.ts(jj, PK)],
                                lhsT=wb_t[:, j, :],
                                rhs=m[:, u * PK:(u + 1) * PK],
                                start=True, stop=True)
                        vh = v[:, t * FC + h * (FC // 2):
                               t * FC + (h + 1) * (FC // 2)]
                        nc.vector.scalar_tensor_tensor(
                            out=vh, in0=vh, scalar=col(c_dprev), in1=mb[:],
                            op0=AL.add, op1=AL.mult)
                    if k == n_flows - 1:
                        nc.vector.tensor_scalar(vchunk, vchunk, col(N_FLOWS),
                                                None, AL.add)
                        nc.sync.dma_start(ot_r[:, t, :], vchunk)

    nc.compile()
    return nc


def _host_params(x0s, alpha_primes, beta_primes, n_flows=N_FLOWS):
    x0s = np.asarray(x0s, np.float32)
    sp_a = np.logaddexp(np.float32(0.0), np.asarray(alpha_primes, np.float32))
    sp_b = np.logaddexp(np.float32(0.0), np.asarray(beta_primes, np.float32))
    alpha = sp_a.astype(np.float32)
    beta = (-alpha + sp_b).astype(np.float32)

    # params: dprev[k], final, invbeta[k], alpha/beta[k], ones
    pr = np.zeros((128, 3 * N_FLOWS + 2), np.float32)
    pr[:, 3 * N_FLOWS + 1] = 1.0
    comp = np.arange(128) // 32  # component index per partition
    for k in range(n_flows):
        dprev = -x0s[0] if k == 0 else x0s[k - 1] - x0s[k]
        bk = beta[k] if beta[k] != 0.0 else np.float32(1e-30)
        pr[:, k] = dprev[comp]
        pr[:, N_FLOWS + 1 + k] = 1.0 / bk
        pr[:, 2 * N_FLOWS + 1 + k] = alpha[k] / bk
    pr[:, N_FLOWS] = x0s[n_flows - 1][comp]

    # stationaries: wr reduce (comp partitions -> packed), wb broadcast (K=32)
    import ml_dtypes
    wr = np.zeros((4, 128, 128), np.float32)
    wb = np.zeros((4, 128, 128), np.float32)
    b = np.arange(NB)
    for j in range(4):
        for c in range(4):
            wr[j, 32 * c + b, 32 * j + b] = 1.0
            wb[j, 32 * j + b, 32 * c + b] = 1.0
    return pr, wr.astype(ml_dtypes.bfloat16), wb


def kernel(X, x0s, alpha_primes, beta_primes):
    from concourse.bass_utils import run_bass_kernel_spmd

    X = np.asarray(X, np.float32)
    pr, wr, wb = _host_params(x0s, alpha_primes, beta_primes)

    if "nc" not in _CACHE:
        _CACHE["nc"] = _build_program()
    nc = _CACHE["nc"]

    in_maps = []
    for c in range(N_CORES):
        shard = X[c * S:(c + 1) * S]
        in_maps.append({
            "xt": np.ascontiguousarray(shard.T),
            "wr": wr,
            "wb": wb,
            "pr": pr,
        })
    res = run_bass_kernel_spmd(nc, in_maps, list(range(N_CORES)))
    out = np.empty((BATCH, DIM), np.float32)
    for c in range(N_CORES):
        out[c * S:(c + 1) * S] = res.results[c]["ot"].T
    return out
